# revision 10
# baseline (speedup 1.0000x reference)
"""Multi-head attention (b=2, t=2048, k=1024, 16 heads) on 8 TRN2 NeuronCores.

Sharding: batch across 2 groups of 4 cores; within a group, heads are
tensor-parallel (4 heads/core, processed as 2 head-pairs).  Per-core pipeline:
  1. fp32r projections from pre-transposed x/W (host supplies x.T, W.T slices)
  2. attention per (head-pair, q-chunk): the two heads share each PE slot at
     full 128x128 array occupancy (keeps the HAM clock gate warm):
       S slot: S.T_A (rows 0-63) + S.T_B (rows 64-127) -> one psum [128, 1024]
       exp [128, 1024] -> P (bf16)
       O slot: col-tiled bf16 matmuls (tile_position (0,0)/(0,64)) accum O.T
       d slot: ones-matmuls producing softmax denominators replicated 64x
     normalize with vector reciprocal + elementwise multiply
  3. per-head-pair 4-core AllGather assembles all 16 heads' O.T
  4. Wo matmul (rows permuted on host to match gather order) -> y.T slice
"""

import sys

if '/opt/trn_rl_repo' not in sys.path:
    sys.path.insert(0, '/opt/trn_rl_repo')

import ml_dtypes
import numpy as np

B = 2
T = 2048
KD = 1024
NH = 16
HS = 64
NCORES = 8
GROUP = 4                 # cores per batch group
NH_LOC = NH // GROUP      # heads per core
NHP = NH_LOC // 2         # head-pairs per core
TSLICE = T // GROUP       # output tokens per core
HFEAT = NH_LOC * HS       # 256 local head features
NKT = T // 128            # 16 key-token tiles
NKD = KD // 128           # 8 model-feature tiles
NQ4 = T // 512            # 4 query chunks of 512

_CACHE = {}


def _build():
    import concourse.bass as bass
    import concourse.mybir as mybir
    import concourse.tile as tile
    from concourse import bacc

    F32 = mybir.dt.float32
    F32R = mybir.dt.float32r
    BF16 = mybir.dt.bfloat16
    AF = mybir.ActivationFunctionType

    nc = bacc.Bacc("TRN2", target_bir_lowering=False, debug=False,
                   num_devices=NCORES)

    xT = nc.dram_tensor("xT", [KD, T], BF16, kind="ExternalInput")
    wqT = nc.dram_tensor("wqT", [KD, HFEAT], BF16, kind="ExternalInput")
    wkT = nc.dram_tensor("wkT", [KD, HFEAT], BF16, kind="ExternalInput")
    wvT = nc.dram_tensor("wvT", [KD, HFEAT], BF16, kind="ExternalInput")
    woT = nc.dram_tensor("woT", [KD, KD], F32R, kind="ExternalInput")
    yT = nc.dram_tensor("yT", [KD, TSLICE], F32, kind="ExternalOutput")

    rgroups = [list(range(GROUP)), list(range(GROUP, 2 * GROUP))]

    with tile.TileContext(nc) as tc:
        with (
            tc.tile_pool(name="qk", bufs=1) as qk_pool,
            tc.tile_pool(name="vpp", bufs=1) as vp_pool,
            tc.tile_pool(name="pt", bufs=3) as pt_pool,
            tc.tile_pool(name="onorm", bufs=3) as on_pool,
            tc.tile_pool(name="rb", bufs=2) as rb_pool,
            tc.tile_pool(name="dram", bufs=1, space="DRAM") as dram,
        ):
            # rows of qt/kt tile hp: 0-63 = head 2hp, 64-127 = head 2hp+1
            qt = [qk_pool.tile([128, T], BF16, name=f"qt{m}", tag=f"qt{m}")
                  for m in range(2)]
            kt = [qk_pool.tile([128, T], BF16, name=f"kt{m}", tag=f"kt{m}")
                  for m in range(2)]
            # V in bf16 token-major: [tok%128, kt_tile, head, 64]
            vp = vp_pool.tile([128, NKT, NH_LOC, HS], BF16)
            ones = vp_pool.tile([128, HS], BF16)
            nc.vector.memset(ones[:], 1.0)

            agin = [[dram.tile([128, TSLICE], F32, name=f"agin{h}_{q}",
                                tag=f"agin{h}_{q}") for q in range(NQ4)]
                    for h in range(NHP)]
            agout = [dram.tile([NQ4, GROUP * 128, TSLICE], F32, name=f"agout{h}",
                               tag=f"agout{h}") for h in range(NHP)]

            # ---- phase 1: projections ----
            with (
                tc.tile_pool(name="xw", bufs=1) as xw_pool,
                tc.tile_pool(name="ppsum", bufs=2, space="PSUM") as ppsum,
            ):
                wq = xw_pool.tile([128, NKD, HFEAT], BF16)
                wk = xw_pool.tile([128, NKD, HFEAT], BF16)
                wv = xw_pool.tile([128, NKD, HFEAT], BF16)
                for wtile, wdram in ((wq, wqT), (wk, wkT), (wv, wvT)):
                    for k in range(NKD):
                        nc.sync.dma_start(
                            wtile[:, k, :], wdram.ap()[128 * k:128 * (k + 1), :])
                xt = xw_pool.tile([128, NKD, T], BF16)
                for k in range(NKD):
                    nc.sync.dma_start(
                        xt[:, k, :], xT.ap()[128 * k:128 * (k + 1), :])

                # Q.T / K.T feature-major [256, T]
                for wtile, dst in ((wq, qt), (wk, kt)):
                    for m in range(2):
                        for n in range(4):
                            ps = ppsum.tile([128, 512], F32, tag="proj")
                            for k in range(NKD):
                                nc.tensor.matmul(
                                    ps[:],
                                    wtile[:, k, 128 * m:128 * (m + 1)],
                                    xt[:, k, 512 * n:512 * (n + 1)],
                                    start=(k == 0), stop=(k == NKD - 1),
                                )
                            nc.vector.tensor_copy(
                                dst[m][:, 512 * n:512 * (n + 1)], ps[:])

                # V token-major -> vp[:, mt, h, :] (bf16)
                for mt in range(NKT):
                    ps = ppsum.tile([128, HFEAT], F32, tag="projv")
                    for k in range(NKD):
                        nc.tensor.matmul(
                            ps[:],
                            xt[:, k, 128 * mt:128 * (mt + 1)],
                            wv[:, k, :],
                            start=(k == 0), stop=(k == NKD - 1),
                        )
                    nc.vector.tensor_copy(
                        vp[:, mt, :, :],
                        ps[:].rearrange("p (h d) -> p h d", h=NH_LOC),
                    )

            # ---- phase 2 + 3 share the wo weights ----
            with tc.tile_pool(name="wo", bufs=1) as wo_pool:
                wo = wo_pool.tile([128, NKD, KD], F32R)
                for k in range(NKD):
                    nc.sync.dma_start(
                        wo[:, k, :], woT.ap()[128 * k:128 * (k + 1), :])

                # ---- phase 2: attention, two heads per PE slot ----
                with (
                    tc.tile_pool(name="spsum", bufs=2, space="PSUM") as spsum,
                    tc.tile_pool(name="opsum", bufs=2, space="PSUM") as opsum,
                ):
                    for hp in range(NHP):
                        hA, hB = 2 * hp, 2 * hp + 1
                        for q4 in range(NQ4):
                            qs = slice(512 * q4, 512 * (q4 + 1))
                            op = opsum.tile([128, 512], F32, tag="op")
                            dp = opsum.tile([128, 512], F32, tag="dp")
                            pts = []

                            def odslot(km, op=op, dp=dp, pts=pts, hA=hA, hB=hB):
                                nc.tensor.matmul(
                                    op[0:64, :], vp[:, km, hA, :],
                                    pts[km][:, 0:512],
                                    start=(km == 0), stop=(km == NKT - 1),
                                    tile_position=(0, 0))
                                nc.tensor.matmul(
                                    op[64:128, :], vp[:, km, hB, :],
                                    pts[km][:, 512:1024],
                                    start=(km == 0), stop=(km == NKT - 1),
                                    tile_position=(0, 64))
                                nc.tensor.matmul(
                                    dp[0:64, :], ones[:],
                                    pts[km][:, 0:512],
                                    start=(km == 0), stop=(km == NKT - 1),
                                    tile_position=(0, 0))
                                nc.tensor.matmul(
                                    dp[64:128, :], ones[:],
                                    pts[km][:, 512:1024],
                                    start=(km == 0), stop=(km == NKT - 1),
                                    tile_position=(0, 64))

                            for ktile in range(NKT):
                                ks = slice(128 * ktile, 128 * (ktile + 1))
                                sp = spsum.tile([128, 1024], F32, tag="sp")
                                nc.tensor.matmul(
                                    sp[:, 0:512], kt[hp][0:64, ks],
                                    qt[hp][0:64, qs], start=True, stop=True)
                                nc.tensor.matmul(
                                    sp[:, 512:1024], kt[hp][64:128, ks],
                                    qt[hp][64:128, qs], start=True, stop=True)
                                pt = pt_pool.tile([128, 1024], BF16, tag="pt")
                                nc.scalar.activation(pt[:], sp[:], AF.Exp,
                                                     scale=0.03125)
                                pts.append(pt)
                                # trail one step behind so PE fills during exp
                                if ktile > 0:
                                    odslot(ktile - 1)
                            odslot(NKT - 1)

                            rb = rb_pool.tile([128, 512], F32, tag="rb")
                            nc.vector.reciprocal(rb[:], dp[:])
                            onorm = on_pool.tile([128, 512], F32, tag="on")
                            nc.vector.tensor_mul(onorm[:], op[:], rb[:])
                            nc.sync.dma_start(agin[hp][q4][:], onorm[:])
                            nc.gpsimd.collective_compute(
                                "AllGather",
                                mybir.AluOpType.bypass,
                                replica_groups=rgroups,
                                ins=[agin[hp][q4].opt()],
                                outs=[agout[hp][q4, :, :]],
                            )

                # ---- phase 3: Wo ----
                with (
                    tc.tile_pool(name="orhs", bufs=1) as orhs_pool,
                    tc.tile_pool(name="yt", bufs=2) as yt_pool,
                    tc.tile_pool(name="ypsum", bufs=1, space="PSUM") as ypsum,
                ):
                    pid = nc.partition_id()
                    rank = pid % GROUP
                    rhs = [orhs_pool.tile([128, TSLICE], F32R, name=f"rhs{k}",
                                          tag=f"rhs{k}") for k in range(NKD)]
                    yps = [ypsum.tile([128, TSLICE], F32, name=f"yp{m}",
                                      tag=f"yp{m}") for m in range(NKD)]

                    def wo_pass(ks):
                        for k in ks:
                            hp, src = k // GROUP, k % GROUP
                            nc.sync.dma_start(
                                rhs[k][:],
                                agout[hp][bass.ds(rank, 1),
                                          128 * src:128 * (src + 1),
                                          :].squeeze(0).bitcast(F32R),
                            )
                        for m in range(NKD):
                            for k in ks:
                                nc.tensor.matmul(
                                    yps[m][:], wo[:, k, 128 * m:128 * (m + 1)],
                                    rhs[k][:],
                                    start=(k == 0), stop=(k == NKD - 1),
                                )

                    # pass A: hp0 k-tiles, runs while the hp1 gathers finish
                    wo_pass(range(GROUP))
                    wo_pass(range(GROUP, NKD))
                    for m in range(NKD):
                        yt_s = yt_pool.tile([128, TSLICE], F32, tag="yt")
                        nc.vector.tensor_copy(yt_s[:], yps[m][:])
                        nc.sync.dma_start(yT.ap()[128 * m:128 * (m + 1), :], yt_s[:])

    nc.compile()
    return nc


def _get_nc():
    if "nc" not in _CACHE:
        _CACHE["nc"] = _build()
    return _CACHE["nc"]


def _make_in_maps(x, Wq, Wk, Wv, Wo):
    # Wo rows permuted to match the AllGather assembly order:
    # feature i = (pair hp, source rank s, head-in-pair a, dim d) -> head 4s+2hp+a
    idx = np.arange(KD)
    hp, rem = idx // (GROUP * 128), idx % (GROUP * 128)
    s, r = rem // 128, rem % 128
    a, d = r // HS, r % HS
    perm = (GROUP * s + 2 * hp + a) * HS + d
    woTp = np.ascontiguousarray(Wo.T[perm])

    in_maps = []
    for c in range(NCORES):
        g, r = c // GROUP, c % GROUP
        rows = slice(r * HFEAT, (r + 1) * HFEAT)
        in_maps.append({
            "xT": np.ascontiguousarray(x[g].T).astype(ml_dtypes.bfloat16),
            "wqT": np.ascontiguousarray(Wq[rows].T).astype(ml_dtypes.bfloat16),
            "wkT": np.ascontiguousarray(Wk[rows].T).astype(ml_dtypes.bfloat16),
            "wvT": np.ascontiguousarray(Wv[rows].T).astype(ml_dtypes.bfloat16),
            "woT": woTp,
        })
    return in_maps


def kernel(x, Wq, Wk, Wv, Wo):
    from concourse import bass_utils

    x = np.asarray(x, dtype=np.float32)
    Wq = np.asarray(Wq, dtype=np.float32)
    Wk = np.asarray(Wk, dtype=np.float32)
    Wv = np.asarray(Wv, dtype=np.float32)
    Wo = np.asarray(Wo, dtype=np.float32)

    nc = _get_nc()
    in_maps = _make_in_maps(x, Wq, Wk, Wv, Wo)
    res = bass_utils.run_bass_kernel_spmd(nc, in_maps, core_ids=list(range(NCORES)))

    out = np.empty((B, T, KD), dtype=np.float32)
    for c in range(NCORES):
        g, r = c // GROUP, c % GROUP
        out[g, r * TSLICE:(r + 1) * TSLICE, :] = res.results[c]["yT"].T
    return out


# revision 11
# speedup vs baseline: 1.0253x; 1.0253x over previous
"""Multi-head attention (b=2, t=2048, k=1024, 16 heads) on 8 TRN2 NeuronCores.

Sharding: batch across 2 groups of 4 cores; within a group, heads are
tensor-parallel (4 heads/core, processed as 2 head-pairs).  Per-core pipeline:
  1. fp32r projections from pre-transposed x/W (host supplies x.T, W.T slices)
  2. attention per (head-pair, q-chunk): the two heads share each PE slot at
     full 128x128 array occupancy (keeps the HAM clock gate warm):
       S slot: S.T_A (rows 0-63) + S.T_B (rows 64-127) -> one psum [128, 1024]
       exp [128, 1024] -> P (bf16)
       O slot: col-tiled bf16 matmuls (tile_position (0,0)/(0,64)) accum O.T
       d slot: ones-matmuls producing softmax denominators replicated 64x
     normalize with vector reciprocal + elementwise multiply
  3. per-head-pair 4-core AllGather assembles all 16 heads' O.T
  4. Wo matmul (rows permuted on host to match gather order) -> y.T slice
"""

import sys

if '/opt/trn_rl_repo' not in sys.path:
    sys.path.insert(0, '/opt/trn_rl_repo')

import ml_dtypes
import numpy as np

B = 2
T = 2048
KD = 1024
NH = 16
HS = 64
NCORES = 8
GROUP = 4                 # cores per batch group
NH_LOC = NH // GROUP      # heads per core
NHP = NH_LOC // 2         # head-pairs per core
TSLICE = T // GROUP       # output tokens per core
HFEAT = NH_LOC * HS       # 256 local head features
NKT = T // 128            # 16 key-token tiles
NKD = KD // 128           # 8 model-feature tiles
NQ4 = T // 512            # 4 query chunks of 512

_CACHE = {}


def _build():
    import concourse.bass as bass
    import concourse.mybir as mybir
    import concourse.tile as tile
    from concourse import bacc

    F32 = mybir.dt.float32
    F32R = mybir.dt.float32r
    BF16 = mybir.dt.bfloat16
    AF = mybir.ActivationFunctionType

    nc = bacc.Bacc("TRN2", target_bir_lowering=False, debug=False,
                   num_devices=NCORES)

    xT = nc.dram_tensor("xT", [KD, T], BF16, kind="ExternalInput")
    wqT = nc.dram_tensor("wqT", [KD, HFEAT], BF16, kind="ExternalInput")
    wkT = nc.dram_tensor("wkT", [KD, HFEAT], BF16, kind="ExternalInput")
    wvT = nc.dram_tensor("wvT", [KD, HFEAT], BF16, kind="ExternalInput")
    woT = nc.dram_tensor("woT", [KD, KD], BF16, kind="ExternalInput")
    yT = nc.dram_tensor("yT", [KD, TSLICE], F32, kind="ExternalOutput")

    rgroups = [list(range(GROUP)), list(range(GROUP, 2 * GROUP))]

    with tile.TileContext(nc) as tc:
        with (
            tc.tile_pool(name="qk", bufs=1) as qk_pool,
            tc.tile_pool(name="vpp", bufs=1) as vp_pool,
            tc.tile_pool(name="pt", bufs=3) as pt_pool,
            tc.tile_pool(name="onorm", bufs=3) as on_pool,
            tc.tile_pool(name="rb", bufs=2) as rb_pool,
            tc.tile_pool(name="dram", bufs=1, space="DRAM") as dram,
        ):
            # rows of qt/kt tile hp: 0-63 = head 2hp, 64-127 = head 2hp+1
            qt = [qk_pool.tile([128, T], BF16, name=f"qt{m}", tag=f"qt{m}")
                  for m in range(2)]
            kt = [qk_pool.tile([128, T], BF16, name=f"kt{m}", tag=f"kt{m}")
                  for m in range(2)]
            # V in bf16 token-major: [tok%128, kt_tile, head, 64]
            vp = vp_pool.tile([128, NKT, NH_LOC, HS], BF16)
            ones = vp_pool.tile([128, HS], BF16)
            nc.vector.memset(ones[:], 1.0)

            agin = [[dram.tile([128, TSLICE], BF16, name=f"agin{h}_{q}",
                                tag=f"agin{h}_{q}") for q in range(NQ4)]
                    for h in range(NHP)]
            agout = [dram.tile([NQ4, GROUP * 128, TSLICE], BF16, name=f"agout{h}",
                               tag=f"agout{h}") for h in range(NHP)]

            # ---- phase 1: projections ----
            with (
                tc.tile_pool(name="xw", bufs=1) as xw_pool,
                tc.tile_pool(name="ppsum", bufs=2, space="PSUM") as ppsum,
            ):
                wq = xw_pool.tile([128, NKD, HFEAT], BF16)
                wk = xw_pool.tile([128, NKD, HFEAT], BF16)
                wv = xw_pool.tile([128, NKD, HFEAT], BF16)
                for wtile, wdram in ((wq, wqT), (wk, wkT), (wv, wvT)):
                    for k in range(NKD):
                        nc.sync.dma_start(
                            wtile[:, k, :], wdram.ap()[128 * k:128 * (k + 1), :])
                xt = xw_pool.tile([128, NKD, T], BF16)
                for k in range(NKD):
                    nc.sync.dma_start(
                        xt[:, k, :], xT.ap()[128 * k:128 * (k + 1), :])

                # Q.T / K.T feature-major [256, T]
                for wtile, dst in ((wq, qt), (wk, kt)):
                    for m in range(2):
                        for n in range(4):
                            ps = ppsum.tile([128, 512], F32, tag="proj")
                            for k in range(NKD):
                                nc.tensor.matmul(
                                    ps[:],
                                    wtile[:, k, 128 * m:128 * (m + 1)],
                                    xt[:, k, 512 * n:512 * (n + 1)],
                                    start=(k == 0), stop=(k == NKD - 1),
                                )
                            nc.vector.tensor_copy(
                                dst[m][:, 512 * n:512 * (n + 1)], ps[:])

                # V token-major -> vp[:, mt, h, :] (bf16)
                for mt in range(NKT):
                    ps = ppsum.tile([128, HFEAT], F32, tag="projv")
                    for k in range(NKD):
                        nc.tensor.matmul(
                            ps[:],
                            xt[:, k, 128 * mt:128 * (mt + 1)],
                            wv[:, k, :],
                            start=(k == 0), stop=(k == NKD - 1),
                        )
                    vcopy = nc.vector.tensor_copy(
                        vp[:, mt, :, :],
                        ps[:].rearrange("p (h d) -> p h d", h=NH_LOC),
                    )

            # ---- phase 2 + 3 share the wo weights ----
            with tc.tile_pool(name="wo", bufs=1) as wo_pool:
                wo = wo_pool.tile([128, NKD, KD], BF16)
                for k in range(NKD):
                    wdma = nc.sync.dma_start(
                        wo[:, k, :], woT.ap()[128 * k:128 * (k + 1), :])
                    tile.add_dep_helper(vcopy.ins, wdma.ins, sync=False,
                                        reason="defer wo prefetch past proj")

                # ---- phase 2: attention, two heads per PE slot ----
                with (
                    tc.tile_pool(name="spsum", bufs=2, space="PSUM") as spsum,
                    tc.tile_pool(name="opsum", bufs=2, space="PSUM") as opsum,
                ):
                    for hp in range(NHP):
                        hA, hB = 2 * hp, 2 * hp + 1
                        for q4 in range(NQ4):
                            qs = slice(512 * q4, 512 * (q4 + 1))
                            op = opsum.tile([128, 512], F32, tag="op")
                            dp = opsum.tile([128, 512], F32, tag="dp")
                            pts = []

                            def odslot(km, op=op, dp=dp, pts=pts, hA=hA, hB=hB):
                                nc.tensor.matmul(
                                    op[0:64, :], vp[:, km, hA, :],
                                    pts[km][:, 0:512],
                                    start=(km == 0), stop=(km == NKT - 1),
                                    tile_position=(0, 0))
                                nc.tensor.matmul(
                                    op[64:128, :], vp[:, km, hB, :],
                                    pts[km][:, 512:1024],
                                    start=(km == 0), stop=(km == NKT - 1),
                                    tile_position=(0, 64))
                                nc.tensor.matmul(
                                    dp[0:64, :], ones[:],
                                    pts[km][:, 0:512],
                                    start=(km == 0), stop=(km == NKT - 1),
                                    tile_position=(0, 0))
                                nc.tensor.matmul(
                                    dp[64:128, :], ones[:],
                                    pts[km][:, 512:1024],
                                    start=(km == 0), stop=(km == NKT - 1),
                                    tile_position=(0, 64))

                            for ktile in range(NKT):
                                ks = slice(128 * ktile, 128 * (ktile + 1))
                                sp = spsum.tile([128, 1024], F32, tag="sp")
                                nc.tensor.matmul(
                                    sp[:, 0:512], kt[hp][0:64, ks],
                                    qt[hp][0:64, qs], start=True, stop=True)
                                nc.tensor.matmul(
                                    sp[:, 512:1024], kt[hp][64:128, ks],
                                    qt[hp][64:128, qs], start=True, stop=True)
                                pt = pt_pool.tile([128, 1024], BF16, tag="pt")
                                nc.scalar.activation(pt[:], sp[:], AF.Exp,
                                                     scale=0.03125)
                                pts.append(pt)
                                # trail one step behind so PE fills during exp
                                if ktile > 0:
                                    odslot(ktile - 1)
                            odslot(NKT - 1)

                            rb = rb_pool.tile([128, 512], F32, tag="rb")
                            nc.vector.reciprocal(rb[:], dp[:])
                            onorm = on_pool.tile([128, 512], BF16, tag="on")
                            nc.vector.tensor_mul(onorm[:], op[:], rb[:])
                            nc.sync.dma_start(agin[hp][q4][:], onorm[:])
                            nc.gpsimd.collective_compute(
                                "AllGather",
                                mybir.AluOpType.bypass,
                                replica_groups=rgroups,
                                ins=[agin[hp][q4].opt()],
                                outs=[agout[hp][q4, :, :]],
                            )

                # ---- phase 3: Wo ----
                with (
                    tc.tile_pool(name="orhs", bufs=1) as orhs_pool,
                    tc.tile_pool(name="yt", bufs=2) as yt_pool,
                    tc.tile_pool(name="ypsum", bufs=1, space="PSUM") as ypsum,
                ):
                    pid = nc.partition_id()
                    rank = pid % GROUP
                    rhs = [orhs_pool.tile([128, TSLICE], BF16, name=f"rhs{k}",
                                          tag=f"rhs{k}") for k in range(NKD)]
                    yps = [ypsum.tile([128, TSLICE], F32, name=f"yp{m}",
                                      tag=f"yp{m}") for m in range(NKD)]

                    def wo_pass(ks):
                        for k in ks:
                            hp, src = k // GROUP, k % GROUP
                            nc.sync.dma_start(
                                rhs[k][:],
                                agout[hp][bass.ds(rank, 1),
                                          128 * src:128 * (src + 1),
                                          :].squeeze(0),
                            )
                        for m in range(NKD):
                            for k in ks:
                                nc.tensor.matmul(
                                    yps[m][:], wo[:, k, 128 * m:128 * (m + 1)],
                                    rhs[k][:],
                                    start=(k == 0), stop=(k == NKD - 1),
                                )

                    # pass A: hp0 k-tiles, runs while the hp1 gathers finish
                    wo_pass(range(GROUP))
                    wo_pass(range(GROUP, NKD))
                    for m in range(NKD):
                        yt_s = yt_pool.tile([128, TSLICE], F32, tag="yt")
                        nc.vector.tensor_copy(yt_s[:], yps[m][:])
                        nc.sync.dma_start(yT.ap()[128 * m:128 * (m + 1), :], yt_s[:])

    nc.compile()
    return nc


def _get_nc():
    if "nc" not in _CACHE:
        _CACHE["nc"] = _build()
    return _CACHE["nc"]


def _make_in_maps(x, Wq, Wk, Wv, Wo):
    # Wo rows permuted to match the AllGather assembly order:
    # feature i = (pair hp, source rank s, head-in-pair a, dim d) -> head 4s+2hp+a
    idx = np.arange(KD)
    hp, rem = idx // (GROUP * 128), idx % (GROUP * 128)
    s, r = rem // 128, rem % 128
    a, d = r // HS, r % HS
    perm = (GROUP * s + 2 * hp + a) * HS + d
    woTp = np.ascontiguousarray(Wo.T[perm]).astype(ml_dtypes.bfloat16)

    in_maps = []
    for c in range(NCORES):
        g, r = c // GROUP, c % GROUP
        rows = slice(r * HFEAT, (r + 1) * HFEAT)
        in_maps.append({
            "xT": np.ascontiguousarray(x[g].T).astype(ml_dtypes.bfloat16),
            "wqT": np.ascontiguousarray(Wq[rows].T).astype(ml_dtypes.bfloat16),
            "wkT": np.ascontiguousarray(Wk[rows].T).astype(ml_dtypes.bfloat16),
            "wvT": np.ascontiguousarray(Wv[rows].T).astype(ml_dtypes.bfloat16),
            "woT": woTp,
        })
    return in_maps


def kernel(x, Wq, Wk, Wv, Wo):
    from concourse import bass_utils

    x = np.asarray(x, dtype=np.float32)
    Wq = np.asarray(Wq, dtype=np.float32)
    Wk = np.asarray(Wk, dtype=np.float32)
    Wv = np.asarray(Wv, dtype=np.float32)
    Wo = np.asarray(Wo, dtype=np.float32)

    nc = _get_nc()
    in_maps = _make_in_maps(x, Wq, Wk, Wv, Wo)
    res = bass_utils.run_bass_kernel_spmd(nc, in_maps, core_ids=list(range(NCORES)))

    out = np.empty((B, T, KD), dtype=np.float32)
    for c in range(NCORES):
        g, r = c // GROUP, c % GROUP
        out[g, r * TSLICE:(r + 1) * TSLICE, :] = res.results[c]["yT"].T
    return out


# revision 12
# speedup vs baseline: 1.1143x; 1.0868x over previous
"""Multi-head attention (b=2, t=2048, k=1024, 16 heads) on 8 TRN2 NeuronCores.

Sharding: batch across 2 groups of 4 cores; within a group, heads are
tensor-parallel (4 heads/core, processed as 2 head-pairs).  Per-core pipeline:
  1. fp32r projections from pre-transposed x/W (host supplies x.T, W.T slices)
  2. attention per (head-pair, q-chunk): the two heads share each PE slot at
     full 128x128 array occupancy (keeps the HAM clock gate warm):
       S slot: S.T_A (rows 0-63) + S.T_B (rows 64-127) -> one psum [128, 1024]
       exp [128, 1024] -> P (bf16)
       O slot: col-tiled bf16 matmuls (tile_position (0,0)/(0,64)) accum O.T
       d slot: ones-matmuls producing softmax denominators replicated 64x
     normalize with vector reciprocal + elementwise multiply
  3. per-head-pair 4-core AllGather assembles all 16 heads' O.T
  4. Wo matmul (rows permuted on host to match gather order) -> y.T slice
"""

import sys

if '/opt/trn_rl_repo' not in sys.path:
    sys.path.insert(0, '/opt/trn_rl_repo')

import ml_dtypes
import numpy as np

B = 2
T = 2048
KD = 1024
NH = 16
HS = 64
NCORES = 8
GROUP = 4                 # cores per batch group
NH_LOC = NH // GROUP      # heads per core
NHP = NH_LOC // 2         # head-pairs per core
TSLICE = T // GROUP       # output tokens per core
HFEAT = NH_LOC * HS       # 256 local head features
NKT = T // 128            # 16 key-token tiles
NKD = KD // 128           # 8 model-feature tiles
NQ4 = T // 512            # 4 query chunks of 512

_CACHE = {}


def _build():
    import concourse.bass as bass
    import concourse.mybir as mybir
    import concourse.tile as tile
    from concourse import bacc

    F32 = mybir.dt.float32
    F32R = mybir.dt.float32r
    BF16 = mybir.dt.bfloat16
    AF = mybir.ActivationFunctionType

    nc = bacc.Bacc("TRN2", target_bir_lowering=False, debug=False,
                   num_devices=NCORES)

    xT = nc.dram_tensor("xT", [KD, T], BF16, kind="ExternalInput")
    wqT = nc.dram_tensor("wqT", [KD, HFEAT], BF16, kind="ExternalInput")
    wkT = nc.dram_tensor("wkT", [KD, HFEAT], BF16, kind="ExternalInput")
    wvT = nc.dram_tensor("wvT", [KD, HFEAT], BF16, kind="ExternalInput")
    woT = nc.dram_tensor("woT", [KD, KD], BF16, kind="ExternalInput")
    yT = nc.dram_tensor("yT", [KD, TSLICE], F32, kind="ExternalOutput")

    rgroups = [list(range(GROUP)), list(range(GROUP, 2 * GROUP))]

    with tile.TileContext(nc) as tc:
        with (
            tc.tile_pool(name="qk", bufs=1) as qk_pool,
            tc.tile_pool(name="vpp", bufs=1) as vp_pool,
            tc.tile_pool(name="pt", bufs=3) as pt_pool,
            tc.tile_pool(name="onorm", bufs=3) as on_pool,
            tc.tile_pool(name="rb", bufs=2) as rb_pool,
            tc.tile_pool(name="dram", bufs=1, space="DRAM") as dram,
        ):
            # rows of qt/kt tile hp: 0-63 = head 2hp, 64-127 = head 2hp+1
            qt = [qk_pool.tile([128, T], BF16, name=f"qt{m}", tag=f"qt{m}")
                  for m in range(2)]
            kt = [qk_pool.tile([128, T], BF16, name=f"kt{m}", tag=f"kt{m}")
                  for m in range(2)]
            # V in bf16 token-major: [tok%128, kt_tile, head, 64]
            vp = vp_pool.tile([128, NKT, NH_LOC, HS], BF16)
            ones = vp_pool.tile([128, HS], BF16)
            nc.vector.memset(ones[:], 1.0)

            agin = [[dram.tile([128, 2 * TSLICE], BF16, name=f"agin{h}_{q}",
                                tag=f"agin{h}_{q}") for q in range(2)]
                    for h in range(NHP)]
            # [4, 256, 1024]: dim0 = 2*qhalf + (row>=256), row-major overall
            agout = [dram.tile([4, 256, 2 * TSLICE], BF16, name=f"agout{h}",
                               tag=f"agout{h}") for h in range(NHP)]

            # ---- phase 1: projections ----
            with (
                tc.tile_pool(name="xw", bufs=1) as xw_pool,
                tc.tile_pool(name="ppsum", bufs=1, space="PSUM") as ppsum,
            ):
                wq = xw_pool.tile([128, NKD, HFEAT], BF16)
                wk = xw_pool.tile([128, NKD, HFEAT], BF16)
                wv = xw_pool.tile([128, NKD, HFEAT], BF16)
                xt = xw_pool.tile([128, NKD, T], BF16)
                for k in range(NKD):
                    for wtile, wdram in ((wq, wqT), (wk, wkT), (wv, wvT)):
                        nc.sync.dma_start(
                            wtile[:, k, :], wdram.ap()[128 * k:128 * (k + 1), :])
                    nc.sync.dma_start(
                        xt[:, k, :], xT.ap()[128 * k:128 * (k + 1), :])

                # Q.T / K.T feature-major [256, T]; k-outer so the PE starts
                # as soon as each xt k-slice lands (8 psum banks accumulate)
                for wtile, dst in ((wq, qt), (wk, kt)):
                    acc = [ppsum.tile([128, 512], F32, name=f"acc{i}",
                                      tag=f"acc{i}") for i in range(8)]
                    for k in range(NKD):
                        for m in range(2):
                            for n in range(4):
                                nc.tensor.matmul(
                                    acc[m * 4 + n][:],
                                    wtile[:, k, 128 * m:128 * (m + 1)],
                                    xt[:, k, 512 * n:512 * (n + 1)],
                                    start=(k == 0), stop=(k == NKD - 1),
                                )
                    for m in range(2):
                        for n in range(4):
                            nc.vector.tensor_copy(
                                dst[m][:, 512 * n:512 * (n + 1)],
                                acc[m * 4 + n][:])

                # V token-major -> vp[:, mt, h, :] (bf16)
                for mt in range(NKT):
                    ps = ppsum.tile([128, HFEAT], F32, tag=f"acc{mt % 8}")
                    for k in range(NKD):
                        nc.tensor.matmul(
                            ps[:],
                            xt[:, k, 128 * mt:128 * (mt + 1)],
                            wv[:, k, :],
                            start=(k == 0), stop=(k == NKD - 1),
                        )
                    vcopy = nc.vector.tensor_copy(
                        vp[:, mt, :, :],
                        ps[:].rearrange("p (h d) -> p h d", h=NH_LOC),
                    )

            # ---- phase 2 + 3 share the wo weights ----
            with tc.tile_pool(name="wo", bufs=1) as wo_pool:
                wo = wo_pool.tile([128, NKD, KD], BF16)
                for k in range(NKD):
                    wdma = nc.sync.dma_start(
                        wo[:, k, :], woT.ap()[128 * k:128 * (k + 1), :])
                    tile.add_dep_helper(vcopy.ins, wdma.ins, sync=False,
                                        reason="defer wo prefetch past proj")

                # ---- phase 2: attention, two heads per PE slot ----
                with (
                    tc.tile_pool(name="spsum", bufs=2, space="PSUM") as spsum,
                    tc.tile_pool(name="opsum", bufs=2, space="PSUM") as opsum,
                ):
                    for hp in range(NHP):
                        hA, hB = 2 * hp, 2 * hp + 1
                        for q4 in range(NQ4):
                            qs = slice(512 * q4, 512 * (q4 + 1))
                            op = opsum.tile([128, 512], F32, tag="op")
                            dp = opsum.tile([128, 512], F32, tag="dp")
                            pts = []

                            def odslot(km, op=op, dp=dp, pts=pts, hA=hA, hB=hB):
                                nc.tensor.matmul(
                                    op[0:64, :], vp[:, km, hA, :],
                                    pts[km][:, 0:512],
                                    start=(km == 0), stop=(km == NKT - 1),
                                    tile_position=(0, 0))
                                nc.tensor.matmul(
                                    op[64:128, :], vp[:, km, hB, :],
                                    pts[km][:, 512:1024],
                                    start=(km == 0), stop=(km == NKT - 1),
                                    tile_position=(0, 64))
                                nc.tensor.matmul(
                                    dp[0:64, :], ones[:],
                                    pts[km][:, 0:512],
                                    start=(km == 0), stop=(km == NKT - 1),
                                    tile_position=(0, 0))
                                nc.tensor.matmul(
                                    dp[64:128, :], ones[:],
                                    pts[km][:, 512:1024],
                                    start=(km == 0), stop=(km == NKT - 1),
                                    tile_position=(0, 64))

                            for ktile in range(NKT):
                                ks = slice(128 * ktile, 128 * (ktile + 1))
                                sp = spsum.tile([128, 1024], F32, tag="sp")
                                nc.tensor.matmul(
                                    sp[:, 0:512], kt[hp][0:64, ks],
                                    qt[hp][0:64, qs], start=True, stop=True)
                                nc.tensor.matmul(
                                    sp[:, 512:1024], kt[hp][64:128, ks],
                                    qt[hp][64:128, qs], start=True, stop=True)
                                pt = pt_pool.tile([128, 1024], BF16, tag="pt")
                                nc.scalar.activation(pt[:], sp[:], AF.Exp,
                                                     scale=0.03125)
                                pts.append(pt)
                                # trail one step behind so PE fills during exp
                                if ktile > 0:
                                    odslot(ktile - 1)
                            odslot(NKT - 1)

                            rb = rb_pool.tile([128, 512], F32, tag="rb")
                            nc.vector.reciprocal(rb[:], dp[:])
                            onorm = on_pool.tile([128, 512], BF16, tag="on")
                            nc.vector.tensor_mul(onorm[:], op[:], rb[:])
                            nc.sync.dma_start(
                                agin[hp][q4 // 2][:, (q4 % 2) * 512:
                                                  (q4 % 2) * 512 + 512],
                                onorm[:])
                            if q4 % 2 == 1:
                                qh = q4 // 2
                                nc.gpsimd.collective_compute(
                                    "AllGather",
                                    mybir.AluOpType.bypass,
                                    replica_groups=rgroups,
                                    ins=[agin[hp][qh].opt()],
                                    outs=[agout[hp][2 * qh:2 * qh + 2, :, :].opt()],
                                )

                # ---- phase 3: Wo ----
                with (
                    tc.tile_pool(name="orhs", bufs=1) as orhs_pool,
                    tc.tile_pool(name="yt", bufs=2) as yt_pool,
                    tc.tile_pool(name="ypsum", bufs=1, space="PSUM") as ypsum,
                ):
                    pid = nc.partition_id()
                    rank2 = pid & 2          # = 2*(rank//2): dim-0 base
                    colo = (pid & 1) * 512   # token-column offset in the q-half
                    rhs = [orhs_pool.tile([128, TSLICE], BF16, name=f"rhs{k}",
                                          tag=f"rhs{k}") for k in range(NKD)]
                    yps = [ypsum.tile([128, TSLICE], F32, name=f"yp{m}",
                                      tag=f"yp{m}") for m in range(NKD)]

                    def wo_pass(ks):
                        for k in ks:
                            hp, src = k // GROUP, k % GROUP
                            d0 = rank2 + (1 if src >= 2 else 0)
                            rows = (src % 2) * 128
                            nc.sync.dma_start(
                                rhs[k][:],
                                agout[hp][bass.ds(d0, 1), rows:rows + 128,
                                          bass.ds(colo, TSLICE)].squeeze(0),
                            )
                        for m in range(NKD):
                            for k in ks:
                                nc.tensor.matmul(
                                    yps[m][:], wo[:, k, 128 * m:128 * (m + 1)],
                                    rhs[k][:],
                                    start=(k == 0), stop=(k == NKD - 1),
                                )

                    # pass A: hp0 k-tiles, runs while the hp1 gathers finish
                    wo_pass(range(GROUP))
                    wo_pass(range(GROUP, NKD))
                    for m in range(NKD):
                        yt_s = yt_pool.tile([128, TSLICE], F32, tag="yt")
                        nc.vector.tensor_copy(yt_s[:], yps[m][:])
                        nc.sync.dma_start(yT.ap()[128 * m:128 * (m + 1), :], yt_s[:])

    nc.compile()
    return nc


def _get_nc():
    if "nc" not in _CACHE:
        _CACHE["nc"] = _build()
    return _CACHE["nc"]


def _make_in_maps(x, Wq, Wk, Wv, Wo):
    # Wo rows permuted to match the AllGather assembly order:
    # feature i = (pair hp, source rank s, head-in-pair a, dim d) -> head 4s+2hp+a
    idx = np.arange(KD)
    hp, rem = idx // (GROUP * 128), idx % (GROUP * 128)
    s, r = rem // 128, rem % 128
    a, d = r // HS, r % HS
    perm = (GROUP * s + 2 * hp + a) * HS + d
    woTp = np.ascontiguousarray(Wo.T[perm]).astype(ml_dtypes.bfloat16)

    in_maps = []
    for c in range(NCORES):
        g, r = c // GROUP, c % GROUP
        rows = slice(r * HFEAT, (r + 1) * HFEAT)
        in_maps.append({
            "xT": np.ascontiguousarray(x[g].T).astype(ml_dtypes.bfloat16),
            "wqT": np.ascontiguousarray(Wq[rows].T).astype(ml_dtypes.bfloat16),
            "wkT": np.ascontiguousarray(Wk[rows].T).astype(ml_dtypes.bfloat16),
            "wvT": np.ascontiguousarray(Wv[rows].T).astype(ml_dtypes.bfloat16),
            "woT": woTp,
        })
    return in_maps


def kernel(x, Wq, Wk, Wv, Wo):
    from concourse import bass_utils

    x = np.asarray(x, dtype=np.float32)
    Wq = np.asarray(Wq, dtype=np.float32)
    Wk = np.asarray(Wk, dtype=np.float32)
    Wv = np.asarray(Wv, dtype=np.float32)
    Wo = np.asarray(Wo, dtype=np.float32)

    nc = _get_nc()
    in_maps = _make_in_maps(x, Wq, Wk, Wv, Wo)
    res = bass_utils.run_bass_kernel_spmd(nc, in_maps, core_ids=list(range(NCORES)))

    out = np.empty((B, T, KD), dtype=np.float32)
    for c in range(NCORES):
        g, r = c // GROUP, c % GROUP
        out[g, r * TSLICE:(r + 1) * TSLICE, :] = res.results[c]["yT"].T
    return out
